# revision 1
# baseline (speedup 1.0000x reference)
"""Trainium2 Bass kernel for the ActionHeadGMM loss.

loss = mean_b sum_k softmax(mix)[b,k] * ( -logN(target_b | mean_bk, diag var_bk) )
with var = 5*sigmoid(cov).

Math used on device (per element, d = mean - target):
  iv  = 1/var = 0.2 + 0.2*exp(-c)         (computed as t2 + 0.2, t2 = exp(-c + ln 0.2))
  lv  = ln(1 + exp(-c)) = ln(5*t2 + 1)    (so ln var = ln 5 - lv)
  -logp[b,k] = C_k + 0.5 * sum_a (d^2*iv - lv),   C_k = 3.5*(ln 2pi + ln 5)
Since sum_k softmax = 1:
  loss = C + (0.5/B) * sum_{b,k,a} softmax(mix)[b,k] * (d^2*iv - lv)

Device computes the big sum (data-parallel over 8 cores, batch-sharded;
total reduction via ones-matmul on the idle TensorEngine, accumulated in
PSUM); host adds the constant and divides, accumulating in float64.
"""

import numpy as np

import concourse.bass as bass
import concourse.tile as tile
from concourse import bacc, mybir
from concourse.bass_utils import run_bass_kernel_spmd
from contextlib import ExitStack, contextmanager


@contextmanager
def _one_act_table():
    """Force insert_act_table_loads to use the one table set that holds
    exp+ln+square ('natural_log_exp_and_others'), so the per-tile
    Exp<->Ln alternation emits a single LoadActFuncSet instead of two
    reloads per tile. Set ids stay canonical (index into act_info.json):
    we only blank the *contents* of the other sets so the greedy
    first-match chooser can't pick them.
    """
    import concourse.bacc as _bacc_mod

    real = _bacc_mod.get_activation_tables
    keep = "natural_log_exp_and_others"

    def patched(arch):
        tables = real(arch)
        if keep not in tables:
            return tables
        return {n: (fns if n == keep else set()) for n, fns in tables.items()}

    _bacc_mod.get_activation_tables = patched
    try:
        yield
    finally:
        _bacc_mod.get_activation_tables = real

P = 128          # SBUF partitions
K = 8            # mixture components
A = 7            # action dim
KA = K * A
N_CORES = 8

LN02 = float(np.log(0.2))
C_CONST = 3.5 * (float(np.log(2.0 * np.pi)) + float(np.log(5.0)))

f32 = mybir.dt.float32
bf16 = mybir.dt.bfloat16
Exp = mybir.ActivationFunctionType.Exp
Ln = mybir.ActivationFunctionType.Ln
Square = mybir.ActivationFunctionType.Square
Alu = mybir.AluOpType
AxX = mybir.AxisListType.X

# feature flags (tuned via TimelineSim + A/B on hardware; winner of the
# final HW sweep: stt-fused q, mixn on GpSimd, d split 50/50 Pool/DVE,
# 2-tile grouped DMA. DVE is the critical engine on HW: anything moved
# off it (iv via stt fusion, mixn to Pool) bought ~2x its sim-model cost)
CFG = dict(
    pe_reduce=True,    # total sum via TensorE ones-matmul (else DVE tensor_reduce)
    d_frac_pool=0.0,   # fraction of the subtract's batch rows done on GpSimd
                       # (0.0: broadcast subtract is pathologically slow on
                       # Q7; DVE at f32 1x still beats it)
    d2_frac_act=1.0,   # fraction of the square op's columns done on ACT
                       # (rest on DVE as d*d tensor_tensor)
    mixn_eng="pool",   # engine for mixn = em * recip(sum): "vec"|"pool"
    t2_bf16=True,      # keep exp(-c+ln.2) in bf16 (iv tensor_scalar hits 4x)
    inplace=True,      # overwrite dead tiles (d2->d, q->iv, e->q, f->e buffers)
    io_bufs=2,
    mid_bufs=2,
    stt_q=True,        # fuse iv+q: q = (t2+0.2)*d2 via scalar_tensor_tensor
    dma_group=2,       # tiles loaded per dma_start
    dma_ring="sync",   # "sync" | "alt" (alternate sync/scalar HWDGE rings)
    dma_only=False,    # ablation: only the loads, no compute
    compute_only=False,  # ablation: no loads, compute on stale SBUF
)

# packed per-tile input layout (host interleaves all four tensors so each
# tile is a single contiguous DMA): [m (bb*KA) | c (bb*KA) | mx (bb*K) |
# tg (bb*A)] f32 per partition, per tile.
PACK = 2 * KA + K + A     # 127 floats per batch row


def build_nc(rows_per_part: int, bb: int, cfg: dict | None = None, reps: int = 1):
    cfg = {**CFG, **(cfg or {})}
    R = rows_per_part
    assert R % bb == 0
    ntiles = R // bb
    F = bb * KA          # elements/partition/tile for [b,k,a] tensors
    Fk = bb * K
    Fa = bb * A
    FP = bb * PACK
    FC = next(c for c in range(min(F, 512), 0, -1) if F % c == 0)
    nchunks = F // FC

    nc = bacc.Bacc("TRN2", target_bir_lowering=False, debug=False)

    # activation float biases require registered const APs
    for val in (LN02,):
        t = nc.alloc_sbuf_tensor(f"const-f32-{val}", [128, 1], f32)
        nc.gpsimd.memset(t.ap(), val)
        nc.const_aps.aps[(f32, val)] = t.ap()
    nc.all_engine_barrier()

    data_d = nc.dram_tensor("data", [P, R * PACK], f32, kind="ExternalInput")
    if cfg["pe_reduce"]:
        out_d = nc.dram_tensor("out", [1, FC], f32, kind="ExternalOutput")
    else:
        out_d = nc.dram_tensor("out", [P, ntiles], f32, kind="ExternalOutput")

    t2dt = bf16 if cfg["t2_bf16"] else f32

    with tile.TileContext(nc) as tc, ExitStack() as exs:
        io = exs.enter_context(tc.tile_pool(name="io", bufs=cfg["io_bufs"]))
        mid = exs.enter_context(tc.tile_pool(name="mid", bufs=cfg["mid_bufs"]))
        accp = exs.enter_context(tc.tile_pool(name="accp", bufs=1))

        if cfg["pe_reduce"]:
            psp = exs.enter_context(tc.tile_pool(name="psum", bufs=1, space="PSUM"))
            psum_full = psp.tile([P, FC], f32)
            psum = psum_full[0:1, :]
            ones = accp.tile([P, 1], bf16)
            nc.gpsimd.memset(ones[:, :], 1.0)
        else:
            acc = accp.tile([P, ntiles], f32)

        G = cfg["dma_group"]
        assert ntiles % G == 0
        io_buf = None
        for rep in range(reps):
          for t in range(ntiles):
            if t % G == 0 and not cfg["compute_only"]:
                io_buf = io.tile([P, G * FP], f32, tag="io")
                ring = nc.sync
                if cfg["dma_ring"] == "alt" and (t // G) % 2 == 1:
                    ring = nc.scalar
                ring.dma_start(
                    out=io_buf[:, :], in_=data_d[:, t * FP:(t + G) * FP])
            elif cfg["compute_only"] and t % G == 0:
                io_buf = io.tile([P, G * FP], f32, tag="io")
                nc.gpsimd.memset(io_buf[:, 0:G * FP:FP], 0.125)
            if cfg["dma_only"]:
                continue
            g = t % G
            d_io = io_buf[:, g * FP:(g + 1) * FP]
            m_t = d_io[:, 0:F]
            c_t = d_io[:, F:2 * F]
            mx_t = d_io[:, 2 * F:2 * F + Fk]
            tg_t = d_io[:, 2 * F + Fk:2 * F + Fk + Fa]

            # covariance side: t2 = 0.2*exp(-c); lv = ln(1+5*t2); iv = t2+0.2
            t2_t = mid.tile([P, F], t2dt, tag="t2")
            lv_t = mid.tile([P, F], bf16, tag="lv")
            nc.scalar.activation(t2_t[:, :], c_t[:, :], Exp, bias=LN02, scale=-1.0)
            nc.scalar.activation(lv_t[:, :], t2_t[:, :], Ln, bias=1.0, scale=5.0)
            if not cfg["stt_q"]:
                iv_t = mid.tile([P, F], bf16, tag="iv")
                nc.vector.tensor_scalar(iv_t[:, :], t2_t[:, :], 0.2, None, Alu.add)

            # d = mean - target (broadcast target over k)
            d_t = mid.tile([P, F], bf16, tag="d")
            m_v = m_t[:, :].rearrange("p (b k a) -> p b k a", b=bb, k=K, a=A)
            tg_v = (
                tg_t[:, :]
                .rearrange("p (b a) -> p b a", b=bb, a=A)
                .unsqueeze(2)
                .broadcast_to([P, bb, K, A])
            )
            d_v = d_t[:, :].rearrange("p (b k a) -> p b k a", b=bb, k=K, a=A)
            fp = cfg["d_frac_pool"]
            bp = int(round(fp * bb / 4)) * 4   # batch rows subtracted on Pool
            if bp > 0:
                nc.gpsimd.tensor_tensor(
                    d_v[:, 0:bp], m_v[:, 0:bp], tg_v[:, 0:bp], Alu.subtract)
            if bp < bb:
                nc.vector.tensor_tensor(
                    d_v[:, bp:bb], m_v[:, bp:bb], tg_v[:, bp:bb], Alu.subtract)

            # q = d^2 * iv ; e = q - lv
            d2_t = d_t if cfg["inplace"] else mid.tile([P, F], bf16, tag="d2")
            fa = cfg["d2_frac_act"]
            ca = int(round(fa * F / 64)) * 64  # columns squared on ACT
            if ca > 0:
                nc.scalar.activation(d2_t[:, 0:ca], d_t[:, 0:ca], Square)
            if ca < F:
                nc.vector.tensor_tensor(
                    d2_t[:, ca:F], d_t[:, ca:F], d_t[:, ca:F], Alu.mult)
            if cfg["stt_q"]:
                # q = (t2 + 0.2) * d2 in one fused op
                q_t = mid.tile([P, F], bf16, tag="iv")
                nc.vector.scalar_tensor_tensor(
                    q_t[:, :], t2_t[:, :], 0.2, d2_t[:, :], Alu.add, Alu.mult)
            else:
                q_t = iv_t if cfg["inplace"] else mid.tile([P, F], bf16, tag="q")
                nc.vector.tensor_tensor(q_t[:, :], d2_t[:, :], iv_t[:, :], Alu.mult)
            e_t = q_t if cfg["inplace"] else mid.tile([P, F], bf16, tag="e")
            nc.vector.tensor_tensor(e_t[:, :], q_t[:, :], lv_t[:, :], Alu.subtract)

            # softmax over k: mixn = exp(mx) / sum_k exp(mx)
            em_t = mid.tile([P, Fk], f32, tag="em")
            nc.scalar.activation(em_t[:, :], mx_t[:, :], Exp)
            s_t = mid.tile([P, bb], f32, tag="s")
            em_v = em_t[:, :].rearrange("p (b k) -> p b k", b=bb, k=K)
            nc.vector.reduce_sum(s_t[:, :], em_v, AxX)
            r_t = mid.tile([P, bb], f32, tag="r")
            nc.vector.reciprocal(r_t[:, :], s_t[:, :])
            mixn_t = mid.tile([P, Fk], f32, tag="mixn")
            mixn_v = mixn_t[:, :].rearrange("p (b k) -> p b k", b=bb, k=K)
            r_v = r_t[:, :].unsqueeze(2).broadcast_to([P, bb, K])
            mixn_e = nc.gpsimd if cfg["mixn_eng"] == "pool" else nc.vector
            mixn_e.tensor_tensor(mixn_v, em_v, r_v, Alu.mult)

            # f = e * mixn (broadcast over a)
            f_t = e_t if cfg["inplace"] else mid.tile([P, F], bf16, tag="f")
            mixn_b = (
                mixn_t[:, :]
                .rearrange("p (b k) -> p b k", b=bb, k=K)
                .unsqueeze(3)
                .broadcast_to([P, bb, K, A])
            )
            f_v = f_t[:, :].rearrange("p (b k a) -> p b k a", b=bb, k=K, a=A)
            e_v = e_t[:, :].rearrange("p (b k a) -> p b k a", b=bb, k=K, a=A)
            nc.vector.tensor_tensor(f_v, e_v, mixn_b, Alu.mult)

            # total-sum reduction
            if cfg["pe_reduce"]:
                for ci in range(nchunks):
                    nc.tensor.matmul(
                        psum[:, :],
                        ones[:, :],
                        f_t[:, ci * FC:(ci + 1) * FC],
                        start=(rep == 0 and t == 0 and ci == 0),
                        stop=(rep == reps - 1 and t == ntiles - 1
                              and ci == nchunks - 1),
                    )
            else:
                nc.vector.tensor_reduce(acc[:, t:t + 1], f_t[:, :], AxX, Alu.add)

        if cfg["dma_only"]:
            osb = accp.tile([1, FC], f32)
            nc.gpsimd.memset(osb[:, :], 0.0)
            nc.sync.dma_start(out=out_d[:, :], in_=osb[:, :])
        elif cfg["pe_reduce"]:
            osb = accp.tile([1, FC], f32)
            nc.vector.tensor_copy(osb[:, :], psum[:, :])
            nc.sync.dma_start(out=out_d[:, :], in_=osb[:, :])
        else:
            nc.sync.dma_start(out=out_d[:, :], in_=acc[:, :])

    with _one_act_table():
        nc.compile()
    return nc


_NC_CACHE: dict = {}


def _get_nc(rows_per_part: int, bb: int):
    key = (rows_per_part, bb)
    if key not in _NC_CACHE:
        _NC_CACHE[key] = build_nc(rows_per_part, bb)
    return _NC_CACHE[key]


def make_in_maps(means, covariances, mixing_coefficients, action_targets):
    B = means.shape[0]
    Bc = B // N_CORES
    R = Bc // P
    bb = _pick_bb(R)
    ntiles = R // bb
    in_maps = []
    for c in range(N_CORES):
        sl = slice(c * Bc, (c + 1) * Bc)
        m3 = np.asarray(means[sl], np.float32).reshape(P, ntiles, bb * KA)
        c3 = np.asarray(covariances[sl], np.float32).reshape(P, ntiles, bb * KA)
        x3 = np.asarray(
            mixing_coefficients[sl], np.float32).reshape(P, ntiles, bb * K)
        t3 = np.asarray(
            action_targets[sl], np.float32).reshape(P, ntiles, bb * A)
        data = np.concatenate([m3, c3, x3, t3], axis=2).reshape(P, R * PACK)
        in_maps.append({"data": np.ascontiguousarray(data)})
    return in_maps


def _pick_bb(R):
    for bb in (64, 32, 16, 8, 4, 2, 1):
        if R % bb == 0:
            return bb
    return 1


def kernel(means, covariances, mixing_coefficients, action_targets):
    B = means.shape[0]
    Bc = B // N_CORES
    R = Bc // P
    bb = _pick_bb(R)
    nc = _get_nc(R, bb)
    in_maps = make_in_maps(means, covariances, mixing_coefficients, action_targets)
    res = run_bass_kernel_spmd(nc, in_maps, core_ids=list(range(N_CORES)))
    total = sum(
        np.asarray(r["out"]).astype(np.float64).sum() for r in res.results
    )
    loss = C_CONST + 0.5 * total / B
    return np.float32(loss)



# revision 3
# speedup vs baseline: 3.0327x; 3.0327x over previous
"""V2 Trainium kernel for ActionHeadGMM loss.

Key changes vs baseline (all driven by measured HW op costs):
  - inputs packed bf16 on host (DMA halves; loss tolerance 2e-2, bf16
    input rounding contributes ~1e-4)
  - w-route: w = (q - lv) * mixn broadcast, summed by the idle
    TensorEngine ones-matmul. No DVE tensor_reduce of F-sized data
    (measured 4.7x model cost) and no scalar_tensor_tensor (4.8x).
  - iv = t2 + 0.2 via tensor_scalar (measured at model speed, 4x
    cheaper than tensor_tensor).
  - e-subtract offloaded to Pool (GpSimd), square split ACT/DVE by
    cfg fraction to balance engines.

Math (per element, d = mean - target):
  t2 = 0.2*exp(-c); iv = 1/var = t2 + 0.2; lv' = ln(1+e^-c) = ln(5t2+1)
  ln var = ln5 - lv'
  loss = C + (0.5/B) * sum_{b,k,a} mixn[b,k] * (d^2*iv - lv')
         C = 3.5*(ln 2pi + ln 5)
"""

import numpy as np

import concourse.bass as bass
import concourse.tile as tile
from concourse import bacc, mybir
from concourse.bass_utils import run_bass_kernel_spmd
from contextlib import ExitStack, contextmanager


@contextmanager
def _one_act_table():
    import concourse.bacc as _bacc_mod

    real = _bacc_mod.get_activation_tables
    keep = "natural_log_exp_and_others"

    def patched(arch):
        tables = real(arch)
        if keep not in tables:
            return tables
        return {n: (fns if n == keep else set()) for n, fns in tables.items()}

    _bacc_mod.get_activation_tables = patched
    try:
        yield
    finally:
        _bacc_mod.get_activation_tables = real

P = 128
K = 8
A = 7
KA = K * A
N_CORES = 8

LN02 = float(np.log(0.2))
C_CONST = 3.5 * (float(np.log(2.0 * np.pi)) + float(np.log(5.0)))

f32 = mybir.dt.float32
bf16 = mybir.dt.bfloat16
NP_BF16 = mybir.dt.np(bf16)
Exp = mybir.ActivationFunctionType.Exp
Ln = mybir.ActivationFunctionType.Ln
Square = mybir.ActivationFunctionType.Square
Alu = mybir.AluOpType
AxX = mybir.AxisListType.X

CFG2 = dict(
    bb=128,           # batch rows per partition per tile
    fa=0.875,         # fraction of square on ACT (rest DVE d*d)
    fe=1.0,           # fraction of e-subtract on Pool (rest DVE)
    G=1,              # tiles per dma_start
    io_bufs=2,
    mid_bufs=2,
    d_swap=True,
    w_swap=False,
    mixn_eng="pool",
    dma_only=False,
    compute_only=False,
)

PACK = 2 * KA + K + A     # 127 bf16 per batch row


def build_nc2(rows_per_part: int, cfg: dict | None = None, reps: int = 1):
    cfg = {**CFG2, **(cfg or {})}
    R = rows_per_part
    bb = cfg["bb"]
    assert R % bb == 0
    ntiles = R // bb
    F = bb * KA
    Fk = bb * K
    Fa = bb * A
    FP = bb * PACK
    FC = next(c for c in range(min(F, 512), 0, -1) if F % c == 0)
    nchunks = F // FC

    nc = bacc.Bacc("TRN2", target_bir_lowering=False, debug=False)

    for val in (LN02,):
        t = nc.alloc_sbuf_tensor(f"const-f32-{val}", [128, 1], f32)
        nc.gpsimd.memset(t.ap(), val)
        nc.const_aps.aps[(f32, val)] = t.ap()
    nc.all_engine_barrier()

    data_d = nc.dram_tensor("data", [P, R * PACK], bf16, kind="ExternalInput")
    out_d = nc.dram_tensor("out", [1, FC], f32, kind="ExternalOutput")

    with tile.TileContext(nc) as tc, ExitStack() as exs:
        io = exs.enter_context(tc.tile_pool(name="io", bufs=cfg["io_bufs"]))
        mid = exs.enter_context(tc.tile_pool(name="mid", bufs=cfg["mid_bufs"]))
        accp = exs.enter_context(tc.tile_pool(name="accp", bufs=1))
        psp = exs.enter_context(tc.tile_pool(name="psum", bufs=1, space="PSUM"))

        psum_full = psp.tile([P, FC], f32)
        psum = psum_full[0:1, :]
        ones = accp.tile([P, 1], bf16)
        nc.gpsimd.memset(ones[:, :], 1.0)

        G = cfg["G"]
        assert ntiles % G == 0
        io_buf = None
        mm_i = 0
        for rep in range(reps):
          for t in range(ntiles):
            if t % G == 0 and not cfg["compute_only"]:
                io_buf = io.tile([P, G * FP], bf16, tag="io")
                nc.sync.dma_start(
                    out=io_buf[:, :], in_=data_d[:, t * FP:(t + G) * FP])
            elif cfg["compute_only"] and t % G == 0:
                io_buf = io.tile([P, G * FP], bf16, tag="io")
                nc.gpsimd.memset(io_buf[:, 0:G * FP:FP], 0.125)
            if cfg["dma_only"]:
                continue
            g = t % G
            d_io = io_buf[:, g * FP:(g + 1) * FP]
            m_t = d_io[:, 0:F]
            c_t = d_io[:, F:2 * F]
            mx_t = d_io[:, 2 * F:2 * F + Fk]
            tg_t = d_io[:, 2 * F + Fk:2 * F + Fk + Fa]

            # covariance branch: t2 = 0.2 e^-c ; lv = ln(1+5 t2) ; iv = t2+0.2
            t2_t = mid.tile([P, F], bf16, tag="t2")
            lv_t = mid.tile([P, F], bf16, tag="lv")
            nc.scalar.activation(
                t2_t[:, :], c_t[:, :], Exp, bias=LN02, scale=-1.0)
            nc.scalar.activation(
                lv_t[:, :], t2_t[:, :], Ln, bias=1.0, scale=5.0)
            iv_t = t2_t  # in place: t2 dead after this
            nc.vector.tensor_scalar(
                iv_t[:, :], t2_t[:, :], 0.2, None, Alu.add)

            # d = mean - target (broadcast over k)
            d_t = mid.tile([P, F], bf16, tag="d")
            m_v = m_t[:, :].rearrange("p (b k a) -> p b k a", b=bb, k=K, a=A)
            tg_v = (
                tg_t[:, :]
                .rearrange("p (b a) -> p b a", b=bb, a=A)
                .unsqueeze(2)
                .broadcast_to([P, bb, K, A])
            )
            d_v = d_t[:, :].rearrange("p (b k a) -> p b k a", b=bb, k=K, a=A)
            if cfg["d_swap"]:
                nc.vector.tensor_tensor(d_v, tg_v, m_v, Alu.subtract)
            else:
                nc.vector.tensor_tensor(d_v, m_v, tg_v, Alu.subtract)

            # d2 = d^2 (in place over d): fa columns on ACT, rest DVE
            d2_t = d_t
            ca = int(round(cfg["fa"] * F / 64)) * 64
            if ca > 0:
                nc.scalar.activation(d2_t[:, 0:ca], d_t[:, 0:ca], Square)
            if ca < F:
                nc.vector.tensor_tensor(
                    d2_t[:, ca:F], d_t[:, ca:F], d_t[:, ca:F], Alu.mult)

            # q = iv * d2 ; e = q - lv (fe columns on Pool, rest DVE)
            q_t = mid.tile([P, F], bf16, tag="q")
            nc.vector.tensor_tensor(q_t[:, :], iv_t[:, :], d2_t[:, :], Alu.mult)
            e_t = q_t
            cb = int(round(cfg["fe"] * F / 64)) * 64
            if cb > 0:
                nc.gpsimd.tensor_tensor(
                    e_t[:, 0:cb], q_t[:, 0:cb], lv_t[:, 0:cb], Alu.subtract)
            if cb < F:
                nc.vector.tensor_tensor(
                    e_t[:, cb:F], q_t[:, cb:F], lv_t[:, cb:F], Alu.subtract)

            # softmax over k: mixn = exp(mx) / sum_k exp(mx)  (bf16)
            em_t = mid.tile([P, Fk], f32, tag="em")
            nc.scalar.activation(em_t[:, :], mx_t[:, :], Exp)
            s_t = mid.tile([P, bb], f32, tag="s")
            em_v = em_t[:, :].rearrange("p (b k) -> p b k", b=bb, k=K)
            nc.vector.reduce_sum(s_t[:, :], em_v, AxX)
            r_t = mid.tile([P, bb], f32, tag="r")
            nc.vector.reciprocal(r_t[:, :], s_t[:, :])
            mixn_t = mid.tile([P, Fk], bf16, tag="mixn")
            mixn_v = mixn_t[:, :].rearrange("p (b k) -> p b k", b=bb, k=K)
            r_v = r_t[:, :].unsqueeze(2).broadcast_to([P, bb, K])
            mixn_eng = nc.gpsimd if cfg["mixn_eng"] == "pool" else nc.vector
            mixn_eng.tensor_tensor(mixn_v, em_v, r_v, Alu.mult)

            # w = e * mixn (broadcast over a); reuse d buffer (dead)
            w_t = d_t
            mixn_b = (
                mixn_t[:, :]
                .rearrange("p (b k) -> p b k", b=bb, k=K)
                .unsqueeze(3)
                .broadcast_to([P, bb, K, A])
            )
            w_v = w_t[:, :].rearrange("p (b k a) -> p b k a", b=bb, k=K, a=A)
            e_v = e_t[:, :].rearrange("p (b k a) -> p b k a", b=bb, k=K, a=A)
            if cfg["w_swap"]:
                nc.vector.tensor_tensor(w_v, mixn_b, e_v, Alu.mult)
            else:
                nc.vector.tensor_tensor(w_v, e_v, mixn_b, Alu.mult)

            for ci in range(nchunks):
                nc.tensor.matmul(
                    psum[:, :],
                    ones[:, :],
                    w_t[:, ci * FC:(ci + 1) * FC],
                    start=(mm_i == 0),
                    stop=(rep == reps - 1 and t == ntiles - 1
                          and ci == nchunks - 1),
                )
                mm_i += 1

        if cfg["dma_only"]:
            osb = accp.tile([1, FC], f32)
            nc.gpsimd.memset(osb[:, :], 0.0)
            nc.sync.dma_start(out=out_d[:, :], in_=osb[:, :])
        else:
            osb = accp.tile([1, FC], f32)
            nc.vector.tensor_copy(osb[:, :], psum[:, :])
            nc.sync.dma_start(out=out_d[:, :], in_=osb[:, :])

    with _one_act_table():
        nc.compile()
    return nc


_NC_CACHE: dict = {}


def _get_nc(rows_per_part: int):
    if rows_per_part not in _NC_CACHE:
        _NC_CACHE[rows_per_part] = build_nc2(rows_per_part)
    return _NC_CACHE[rows_per_part]


def make_in_maps2(means, covariances, mixing_coefficients, action_targets,
                  bb=None):
    bb = bb or CFG2["bb"]
    B = means.shape[0]
    Bc = B // N_CORES
    R = Bc // P
    ntiles = R // bb
    in_maps = []
    for c in range(N_CORES):
        sl = slice(c * Bc, (c + 1) * Bc)
        m3 = np.asarray(means[sl], np.float32).reshape(P, ntiles, bb * KA)
        c3 = np.asarray(covariances[sl], np.float32).reshape(P, ntiles, bb * KA)
        x3 = np.asarray(
            mixing_coefficients[sl], np.float32).reshape(P, ntiles, bb * K)
        t3 = np.asarray(
            action_targets[sl], np.float32).reshape(P, ntiles, bb * A)
        data = np.concatenate([m3, c3, x3, t3], axis=2).reshape(P, R * PACK)
        in_maps.append({"data": np.ascontiguousarray(data.astype(NP_BF16))})
    return in_maps


def kernel(means, covariances, mixing_coefficients, action_targets):
    B = means.shape[0]
    Bc = B // N_CORES
    R = Bc // P
    nc = _get_nc(R)
    in_maps = make_in_maps2(
        means, covariances, mixing_coefficients, action_targets)
    res = run_bass_kernel_spmd(nc, in_maps, core_ids=list(range(N_CORES)))
    total = sum(
        np.asarray(r["out"]).astype(np.float64).sum() for r in res.results
    )
    loss = C_CONST + 0.5 * total / B
    return np.float32(loss)


# revision 4
# speedup vs baseline: 3.1703x; 1.0454x over previous
"""V2 Trainium kernel for ActionHeadGMM loss.

Key changes vs baseline (all driven by measured HW op costs):
  - inputs packed bf16 on host (DMA halves; loss tolerance 2e-2, bf16
    input rounding contributes ~1e-4)
  - w-route: w = (q - lv) * mixn broadcast, summed by the idle
    TensorEngine ones-matmul. No DVE tensor_reduce of F-sized data
    (measured 4.7x model cost) and no scalar_tensor_tensor (4.8x).
  - iv = t2 + 0.2 via tensor_scalar (measured at model speed, 4x
    cheaper than tensor_tensor).
  - e-subtract offloaded to Pool (GpSimd), square split ACT/DVE by
    cfg fraction to balance engines.

Math (per element, d = mean - target):
  t2 = 0.2*exp(-c); iv = 1/var = t2 + 0.2; lv' = ln(1+e^-c) = ln(5t2+1)
  ln var = ln5 - lv'
  loss = C + (0.5/B) * sum_{b,k,a} mixn[b,k] * (d^2*iv - lv')
         C = 3.5*(ln 2pi + ln 5)
"""

import numpy as np

import concourse.bass as bass
import concourse.tile as tile
from concourse import bacc, mybir
from concourse.bass_utils import run_bass_kernel_spmd
from contextlib import ExitStack, contextmanager


@contextmanager
def _one_act_table():
    import concourse.bacc as _bacc_mod

    real = _bacc_mod.get_activation_tables
    keep = "natural_log_exp_and_others"

    def patched(arch):
        tables = real(arch)
        if keep not in tables:
            return tables
        return {n: (fns if n == keep else set()) for n, fns in tables.items()}

    _bacc_mod.get_activation_tables = patched
    try:
        yield
    finally:
        _bacc_mod.get_activation_tables = real

P = 128
K = 8
A = 7
KA = K * A
N_CORES = 8

LN02 = float(np.log(0.2))
C_CONST = 3.5 * (float(np.log(2.0 * np.pi)) + float(np.log(5.0)))

f32 = mybir.dt.float32
bf16 = mybir.dt.bfloat16
NP_BF16 = mybir.dt.np(bf16)
Exp = mybir.ActivationFunctionType.Exp
Ln = mybir.ActivationFunctionType.Ln
Square = mybir.ActivationFunctionType.Square
Alu = mybir.AluOpType
AxX = mybir.AxisListType.X

CFG2 = dict(
    bb=64,            # batch rows per partition per tile
    fa=0.875,         # fraction of square on ACT (rest DVE d*d)
    fe=1.0,           # fraction of e-subtract on Pool (rest DVE)
    G=2,              # tiles per dma_start
    io_bufs=2,
    mid_bufs=3,
    d_swap=True,
    w_swap=False,
    mixn_eng="pool",
    dma_only=False,
    compute_only=False,
)

PACK = 2 * KA + K + A     # 127 bf16 per batch row


def build_nc2(rows_per_part: int, cfg: dict | None = None, reps: int = 1):
    cfg = {**CFG2, **(cfg or {})}
    R = rows_per_part
    bb = cfg["bb"]
    assert R % bb == 0
    ntiles = R // bb
    F = bb * KA
    Fk = bb * K
    Fa = bb * A
    FP = bb * PACK
    FC = next(c for c in range(min(F, 512), 0, -1) if F % c == 0)
    nchunks = F // FC

    nc = bacc.Bacc("TRN2", target_bir_lowering=False, debug=False)

    for val in (LN02,):
        t = nc.alloc_sbuf_tensor(f"const-f32-{val}", [128, 1], f32)
        nc.gpsimd.memset(t.ap(), val)
        nc.const_aps.aps[(f32, val)] = t.ap()
    nc.all_engine_barrier()

    data_d = nc.dram_tensor("data", [P, R * PACK], bf16, kind="ExternalInput")
    out_d = nc.dram_tensor("out", [1, FC], f32, kind="ExternalOutput")

    with tile.TileContext(nc) as tc, ExitStack() as exs:
        io = exs.enter_context(tc.tile_pool(name="io", bufs=cfg["io_bufs"]))
        mid = exs.enter_context(tc.tile_pool(name="mid", bufs=cfg["mid_bufs"]))
        accp = exs.enter_context(tc.tile_pool(name="accp", bufs=1))
        psp = exs.enter_context(tc.tile_pool(name="psum", bufs=1, space="PSUM"))

        psum_full = psp.tile([P, FC], f32)
        psum = psum_full[0:1, :]
        ones = accp.tile([P, 1], bf16)
        nc.gpsimd.memset(ones[:, :], 1.0)

        G = cfg["G"]
        assert ntiles % G == 0
        io_buf = None
        mm_i = 0
        for rep in range(reps):
          for t in range(ntiles):
            if t % G == 0 and not cfg["compute_only"]:
                io_buf = io.tile([P, G * FP], bf16, tag="io")
                nc.sync.dma_start(
                    out=io_buf[:, :], in_=data_d[:, t * FP:(t + G) * FP])
            elif cfg["compute_only"] and t % G == 0:
                io_buf = io.tile([P, G * FP], bf16, tag="io")
                nc.gpsimd.memset(io_buf[:, 0:G * FP:FP], 0.125)
            if cfg["dma_only"]:
                continue
            g = t % G
            d_io = io_buf[:, g * FP:(g + 1) * FP]
            m_t = d_io[:, 0:F]
            c_t = d_io[:, F:2 * F]
            mx_t = d_io[:, 2 * F:2 * F + Fk]
            tg_t = d_io[:, 2 * F + Fk:2 * F + Fk + Fa]

            # covariance branch: t2 = 0.2 e^-c ; lv = ln(1+5 t2) ; iv = t2+0.2
            t2_t = mid.tile([P, F], bf16, tag="t2")
            lv_t = mid.tile([P, F], bf16, tag="lv")
            nc.scalar.activation(
                t2_t[:, :], c_t[:, :], Exp, bias=LN02, scale=-1.0)
            nc.scalar.activation(
                lv_t[:, :], t2_t[:, :], Ln, bias=1.0, scale=5.0)
            iv_t = t2_t  # in place: t2 dead after this
            nc.vector.tensor_scalar(
                iv_t[:, :], t2_t[:, :], 0.2, None, Alu.add)

            # d = mean - target (broadcast over k)
            d_t = mid.tile([P, F], bf16, tag="d")
            m_v = m_t[:, :].rearrange("p (b k a) -> p b k a", b=bb, k=K, a=A)
            tg_v = (
                tg_t[:, :]
                .rearrange("p (b a) -> p b a", b=bb, a=A)
                .unsqueeze(2)
                .broadcast_to([P, bb, K, A])
            )
            d_v = d_t[:, :].rearrange("p (b k a) -> p b k a", b=bb, k=K, a=A)
            if cfg["d_swap"]:
                nc.vector.tensor_tensor(d_v, tg_v, m_v, Alu.subtract)
            else:
                nc.vector.tensor_tensor(d_v, m_v, tg_v, Alu.subtract)

            # d2 = d^2 (in place over d): fa columns on ACT, rest DVE
            d2_t = d_t
            ca = int(round(cfg["fa"] * F / 64)) * 64
            if ca > 0:
                nc.scalar.activation(d2_t[:, 0:ca], d_t[:, 0:ca], Square)
            if ca < F:
                nc.vector.tensor_tensor(
                    d2_t[:, ca:F], d_t[:, ca:F], d_t[:, ca:F], Alu.mult)

            # q = iv * d2 ; e = q - lv (fe columns on Pool, rest DVE)
            q_t = mid.tile([P, F], bf16, tag="q")
            nc.vector.tensor_tensor(q_t[:, :], iv_t[:, :], d2_t[:, :], Alu.mult)
            e_t = q_t
            cb = int(round(cfg["fe"] * F / 64)) * 64
            if cb > 0:
                nc.gpsimd.tensor_tensor(
                    e_t[:, 0:cb], q_t[:, 0:cb], lv_t[:, 0:cb], Alu.subtract)
            if cb < F:
                nc.vector.tensor_tensor(
                    e_t[:, cb:F], q_t[:, cb:F], lv_t[:, cb:F], Alu.subtract)

            # softmax over k: mixn = exp(mx) / sum_k exp(mx)  (bf16)
            em_t = mid.tile([P, Fk], f32, tag="em")
            nc.scalar.activation(em_t[:, :], mx_t[:, :], Exp)
            s_t = mid.tile([P, bb], f32, tag="s")
            em_v = em_t[:, :].rearrange("p (b k) -> p b k", b=bb, k=K)
            nc.vector.reduce_sum(s_t[:, :], em_v, AxX)
            r_t = mid.tile([P, bb], f32, tag="r")
            nc.vector.reciprocal(r_t[:, :], s_t[:, :])
            mixn_t = mid.tile([P, Fk], bf16, tag="mixn")
            mixn_v = mixn_t[:, :].rearrange("p (b k) -> p b k", b=bb, k=K)
            r_v = r_t[:, :].unsqueeze(2).broadcast_to([P, bb, K])
            mixn_eng = nc.gpsimd if cfg["mixn_eng"] == "pool" else nc.vector
            mixn_eng.tensor_tensor(mixn_v, em_v, r_v, Alu.mult)

            # w = e * mixn (broadcast over a); reuse d buffer (dead)
            w_t = d_t
            mixn_b = (
                mixn_t[:, :]
                .rearrange("p (b k) -> p b k", b=bb, k=K)
                .unsqueeze(3)
                .broadcast_to([P, bb, K, A])
            )
            w_v = w_t[:, :].rearrange("p (b k a) -> p b k a", b=bb, k=K, a=A)
            e_v = e_t[:, :].rearrange("p (b k a) -> p b k a", b=bb, k=K, a=A)
            if cfg["w_swap"]:
                nc.vector.tensor_tensor(w_v, mixn_b, e_v, Alu.mult)
            else:
                nc.vector.tensor_tensor(w_v, e_v, mixn_b, Alu.mult)

            for ci in range(nchunks):
                nc.tensor.matmul(
                    psum[:, :],
                    ones[:, :],
                    w_t[:, ci * FC:(ci + 1) * FC],
                    start=(mm_i == 0),
                    stop=(rep == reps - 1 and t == ntiles - 1
                          and ci == nchunks - 1),
                )
                mm_i += 1

        if cfg["dma_only"]:
            osb = accp.tile([1, FC], f32)
            nc.gpsimd.memset(osb[:, :], 0.0)
            nc.sync.dma_start(out=out_d[:, :], in_=osb[:, :])
        else:
            osb = accp.tile([1, FC], f32)
            nc.vector.tensor_copy(osb[:, :], psum[:, :])
            nc.sync.dma_start(out=out_d[:, :], in_=osb[:, :])

    with _one_act_table():
        nc.compile()
    return nc


_NC_CACHE: dict = {}


def _get_nc(rows_per_part: int):
    if rows_per_part not in _NC_CACHE:
        _NC_CACHE[rows_per_part] = build_nc2(rows_per_part)
    return _NC_CACHE[rows_per_part]


def make_in_maps2(means, covariances, mixing_coefficients, action_targets,
                  bb=None):
    bb = bb or CFG2["bb"]
    B = means.shape[0]
    Bc = B // N_CORES
    R = Bc // P
    ntiles = R // bb
    in_maps = []
    for c in range(N_CORES):
        sl = slice(c * Bc, (c + 1) * Bc)
        m3 = np.asarray(means[sl], np.float32).reshape(P, ntiles, bb * KA)
        c3 = np.asarray(covariances[sl], np.float32).reshape(P, ntiles, bb * KA)
        x3 = np.asarray(
            mixing_coefficients[sl], np.float32).reshape(P, ntiles, bb * K)
        t3 = np.asarray(
            action_targets[sl], np.float32).reshape(P, ntiles, bb * A)
        data = np.concatenate([m3, c3, x3, t3], axis=2).reshape(P, R * PACK)
        in_maps.append({"data": np.ascontiguousarray(data.astype(NP_BF16))})
    return in_maps


def kernel(means, covariances, mixing_coefficients, action_targets):
    B = means.shape[0]
    Bc = B // N_CORES
    R = Bc // P
    nc = _get_nc(R)
    in_maps = make_in_maps2(
        means, covariances, mixing_coefficients, action_targets)
    res = run_bass_kernel_spmd(nc, in_maps, core_ids=list(range(N_CORES)))
    total = sum(
        np.asarray(r["out"]).astype(np.float64).sum() for r in res.results
    )
    loss = C_CONST + 0.5 * total / B
    return np.float32(loss)
